# revision 27
# baseline (speedup 1.0000x reference)
"""PatternMemory kernel for 8 Trainium2 NeuronCores.

Math (B=8, T=1024, C=1024, P=100):
  ctx_h = context @ W1[:C]                   (B, C)
  trg_h = triggers @ W1[C:]                  (P, C)
  h = relu(ctx_h[:,None,:] + trg_h[None,:,:] + b1)
  logits = h @ W2 + b2[0]                    (B, P)
  scores = sigmoid(logits).mean(axis=0)      (P,)
  w = where(scores > 0.5, scores * conf, 0)
  out = attention_scores + 0.1 * einsum("p,pij->ij", w, biases)

Sharding: core r owns rows [128r, 128(r+1)) of the (T, T) plane. The
(P, T, T) biases tensor — the only big input — splits cleanly along
rows, so every core does the full (cheap) MLP redundantly and there
are no collectives.

Layout (TimelineSim: 197.6us vs 214.1us for the single-stream
baseline; the DMA queue is gapless at the ~358 GB/s HBM bound, so
this is the traffic floor of 69.7MB/core +- startup/drain):
- The whole stream is COLUMN-SPLIT into left (cols 0:512) and right
  (cols 512:1024) halves. Sync-ring DMA order: packed consts (2
  DMAs) -> W1 (32 x [128,512]) -> 100 bias-left -> [16 bias-right,
  then 8 attn-left] -> rest of bias-right -> 8 attn-right. The left
  half's tail (merges + adds + stores) hides under the right half's
  load stream instead of being an exposed ~17us tail at the end.
- W1 tiles live in the SAME rotating pool as the stream tiles, so
  after phase A consumes them their 8.4MB recycles as stream
  buffering (needed while the fp32 MLP computes the weights; fp32r
  was measured at tf32-level error - unusable, the error budget vs
  the 2e-2 rel gate is ~1e-6).
- ctx_h is computed with W1 as the STATIONARY operand and context as
  the 8-column moving rhs (64 small matmuls + 8 rank-1 b1 folds):
  PE matmul cost scales with the moving dim, so this is ~2x cheaper
  than streaming W1 through, and it yields ctx_hT directly in [c, b]
  layout for the per-b relu bias. trg_h keeps the W1-moving dataflow
  in 256-col PSUM groups so phase_b pipelines into the trg stream.
- The weighted accumulation is split across three engines so no
  single one gates the stream: chain 0 (p 0-24) runs as an in-place
  ACT scale (w*b, private SBUF ports) + Pool tensor_add; chains 1-3
  are DVE scalar_tensor_tensor. DVE starts ~36us late (the fp32 MLP
  produces w at ~63us) and its ~0.69us/half-tile cadence never
  catches back up if it owns all 200 half-planes.
- The left-half stores issue from the SWDGE (gpsimd) ring: on the
  HWDGE ring they head-of-line block the right-half loads while
  waiting for the left adds. Right stores are HWDGE (scalar ring).
- Element-wise float op order matches the reference/baseline: same
  4 chains of 25, same merge tree ((0+1),(2+3),(0+2)); mask compare
  folded to ssum > B*thr (exact: x0.125 is a power-of-2 scale) and
  conf*(LAMBDA/B) precomputed off-path (bit-identical for conf=1).
"""

import numpy as np
import bass_rust

from concourse import bass, mybir
from concourse.bass_utils import run_bass_kernel_spmd
from concourse.tile import TileContext

B, T, C, P = 8, 1024, 1024, 100
NCORES = 8
ROWS = T // NCORES  # 128 rows of the (T, T) plane per core
HALF = T // 2
FP32 = mybir.dt.float32
AF = mybir.ActivationFunctionType
ALU = mybir.AluOpType

SIM_THRESHOLD = 0.5
LAMBDA = 0.1

STREAM_BUFS = 82    # rotating [128,512] slots (32 W1 + bias/attn stream)
CHAIN = 25          # patterns per accumulation chain (4 chains)

_NC_CACHE = {}


def _build_nc() -> bass.Bass:
    nc = bass.Bass("TRN2", target_bir_lowering=False, debug=False,
                   num_devices=NCORES)

    bias_s = nc.dram_tensor("bias_s", (P, ROWS, T), FP32, kind="ExternalInput").ap()
    attn_s = nc.dram_tensor("attn_s", (B, ROWS, T), FP32, kind="ExternalInput").ap()
    # packed W1 halves: [r, ch*4096 + kt*512 + c'] = W1[off + kt*128 + r, ch*512 + c']
    w1hi = nc.dram_tensor("w1hi", (128, 8192), FP32, kind="ExternalInput").ap()
    w1lo = nc.dram_tensor("w1lo", (128, 8192), FP32, kind="ExternalInput").ap()
    # packed 128-partition consts: trigp | ctxp | w2r
    constsA = nc.dram_tensor("constsA", (128, 8 * P + 8 * B + 8), FP32,
                             kind="ExternalInput").ap()
    # packed 1-partition consts: b1row | conf | b2
    constsB = nc.dram_tensor("constsB", (1, C + P + 1), FP32,
                             kind="ExternalInput").ap()
    out_s = nc.dram_tensor("out_s", (B, ROWS, T), FP32, kind="ExternalOutput").ap()

    with TileContext(nc) as tc:
        with tc.tile_pool(name="const", bufs=1) as const_pool, \
             tc.tile_pool(name="mlp", bufs=1) as mlp_pool, \
             tc.tile_pool(name="rot", bufs=2) as rot_pool, \
             tc.tile_pool(name="small", bufs=1) as small_pool, \
             tc.tile_pool(name="psA", bufs=1, space="PSUM") as psA, \
             tc.tile_pool(name="psB", bufs=1, space="PSUM") as psB, \
             tc.tile_pool(name="strm", bufs=STREAM_BUFS) as strm_pool, \
             tc.tile_pool(name="accp", bufs=1) as acc_pool:

            # ---- packed consts first on the Sync queue (2 DMAs) ----
            catile = const_pool.tile([128, 8 * P + 8 * B + 8], FP32,
                                     tag="constsA", name="catile")
            nc.sync.dma_start(out=catile, in_=constsA)
            cbtile = const_pool.tile([1, C + P + 1], FP32, tag="constsB",
                                     name="cbtile")
            nc.sync.dma_start(out=cbtile, in_=constsB)
            trigt = catile[:, 0:8 * P]
            ctxt = catile[:, 8 * P:8 * P + 8 * B]
            w2t = catile[:, 8 * P + 8 * B:8 * P + 8 * B + 8]
            b1t = cbtile[:, 0:C]
            conft = cbtile[:, C:C + P]
            b2t = cbtile[:, C + P:C + P + 1]
            # conf * (LAMBDA/B), off the critical path (fp32(LAMBDA/B) ==
            # fp32(LAMBDA) * (1/B) bit-exactly, so w matches the two-step
            # scale of the reference pipeline for conf == 1)
            conft2 = small_pool.tile([1, P], FP32, tag="conft2", name="conft2")
            nc.vector.tensor_scalar_mul(out=conft2, in0=conft,
                                        scalar1=LAMBDA / B)

            # ---- W1 as 32 [128,512] tiles in the shared stream pool.
            # hi (trg) halves for BOTH ch first — trg matmuls are the
            # expensive PE work, unblock them ASAP; cheap ctx waits ----
            w1_tiles = {}
            for half_idx, src in ((0, w1hi), (1, w1lo)):
                for ch in range(2):
                    for kt in range(8):
                        csl = slice(ch * 4096 + kt * 512,
                                    ch * 4096 + (kt + 1) * 512)
                        wt = strm_pool.tile([128, HALF], FP32, tag="strm",
                                            name=f"w1_{ch}_{half_idx}_{kt}")
                        nc.sync.dma_start(out=wt, in_=src[:, csl])
                        w1_tiles[(ch, half_idx, kt)] = wt

            def w1_slice(ch, half_idx, kt):
                return w1_tiles[(ch, half_idx, kt)]

            # ---- full column-split bias stream on the sync ring. The
            # attention planes are NOT separately loaded: each out-tile is
            # an ACT copy of the finished accumulator, and the attn load
            # lands on it with a CCE-ADD SWDGE DMA (gpsimd ring), so the
            # adds never touch the Vector engine and the waiting loads
            # never block the bias stream. Out-tiles are CREATED here (mid
            # rotation) so their pool slots recycle from early-stream
            # tiles, but copy/accum/store are emitted in the tail ----
            bias_tiles = {}
            attns = {}
            ATTN_DELAY = 16   # left attn loads sit this far into the right
            for h in range(2):  # stream (needed only at the left tail)
                csl = slice(h * HALF, (h + 1) * HALF)
                for p in range(P):
                    bt = strm_pool.tile([128, HALF], FP32, tag="strm",
                                        name=f"bias{h}_{p}")
                    nc.sync.dma_start(out=bt, in_=bias_s[p][:, csl])
                    bias_tiles[(h, p)] = bt
                    if h == 1 and p == ATTN_DELAY:
                        for b in range(B):
                            at = strm_pool.tile([128, HALF], FP32, tag="strm",
                                                name=f"attn0_{b}")
                            nc.sync.dma_start(out=at, in_=attn_s[b][:, 0:HALF])
                            attns[(0, b)] = at
            for b in range(B):
                at = strm_pool.tile([128, HALF], FP32, tag="strm",
                                    name=f"attn1_{b}")
                nc.sync.dma_start(out=at, in_=attn_s[b][:, HALF:T])
                attns[(1, b)] = at

            ones = const_pool.tile([1, 128], FP32, tag="ones", name="ones")
            nc.vector.memset(ones, 1.0)
            ident = const_pool.tile([128, 128], FP32, tag="ident", name="ident")
            from concourse.masks import make_identity
            make_identity(nc, ident)

            # ---- phase A-trg: trg_hT [p, c] with W1hi moving ----
            ps_trg = psA.tile([P, C], FP32, tag="trg", name="ps_trg")
            trg_hsb = mlp_pool.tile([P, C], FP32, tag="trg_hsb", name="trg_hsb")
            log_a = psB.tile([1, 512], FP32, tag="log_a", name="log_a")
            log_b = psB.tile([1, 288], FP32, tag="log_b", name="log_b")

            def phase_a_trg(ch):
                # 256-col PSUM groups: the first group of a ch completes 8
                # matmuls earlier, so phase_b overlaps the trg stream
                for g in range(2):
                    osl = slice(ch * 512 + g * 256, ch * 512 + (g + 1) * 256)
                    gsub = slice(g * 256, (g + 1) * 256)
                    for kt in range(8):
                        nc.tensor.matmul(ps_trg[:, osl],
                                         lhsT=trigt[:, kt * P:(kt + 1) * P],
                                         rhs=w1_slice(ch, 0, kt)[:, gsub],
                                         start=(kt == 0), stop=(kt == 7))
                    nc.scalar.activation(out=trg_hsb[:, osl],
                                         in_=ps_trg[:, osl], func=AF.Copy)

            # ---- phase A-ctx: ctx_hT [c, b] with W1lo STATIONARY and
            # context moving (8-col rhs -> cheap), b1 folded via rank-1 ----
            ps_ctxT = psA.tile([128, 8 * B], FP32, tag="ctxT", name="ps_ctxT")

            def phase_a_ctx(ch):
                for cb in range(4):
                    ct = ch * 4 + cb
                    for kt in range(8):
                        wt = w1_slice(ch, 1, kt)
                        nc.tensor.matmul(ps_ctxT[:, ct * B:(ct + 1) * B],
                                         lhsT=wt[:, cb * 128:(cb + 1) * 128],
                                         rhs=ctxt[:, kt * B:(kt + 1) * B],
                                         start=(kt == 0), stop=False)
                    # fold b1 in: ctx_hT[c, b] += b1[c] * 1
                    nc.tensor.matmul(ps_ctxT[:, ct * B:(ct + 1) * B],
                                     lhsT=b1t[0:1, ct * 128:(ct + 1) * 128],
                                     rhs=ones[0:1, 0:B], start=False, stop=True)

            def phase_b(ct):
                csl = slice(ct * 128, (ct + 1) * 128)
                t2 = psB.tile([128, P], FP32, tag="t2", bufs=2, name=f"t2_{ct}")
                nc.tensor.transpose(t2, trg_hsb[:, csl], ident[0:P, 0:P])
                bvs = rot_pool.tile([128, B], FP32, tag="bvs", name=f"bvs_{ct}")
                nc.scalar.activation(out=bvs, func=AF.Copy,
                                     in_=ps_ctxT[:, ct * B:(ct + 1) * B])
                hT = rot_pool.tile([128, B * P], FP32, tag="hT", name=f"hT_{ct}")
                # all-ACT relu: keep the Vector engine free for the bias
                # stream (ACT has its own SBUF ports)
                for b in range(B):
                    nc.scalar.activation(out=hT[:, b * P:(b + 1) * P], in_=t2,
                                         func=AF.Relu, bias=bvs[:, b:b + 1])
                nc.tensor.matmul(log_a, lhsT=w2t[:, ct:ct + 1],
                                 rhs=hT[:, 0:512],
                                 start=(ct == 0), stop=(ct == 7))
                nc.tensor.matmul(log_b, lhsT=w2t[:, ct:ct + 1],
                                 rhs=hT[:, 512:800],
                                 start=(ct == 0), stop=(ct == 7))

            phase_a_trg(0)
            phase_a_ctx(0)
            for ct in range(4):
                phase_b(ct)
            phase_a_trg(1)
            phase_a_ctx(1)
            for ct in range(4, 8):
                phase_b(ct)

            # ---- scores -> weights (tiny, [1, *] on one partition) ----
            sig = small_pool.tile([1, B * P], FP32, tag="sig", name="sig")
            nc.scalar.activation(out=sig[:, 0:512], in_=log_a,
                                 func=AF.Sigmoid, bias=b2t[:, 0:1])
            nc.scalar.activation(out=sig[:, 512:800], in_=log_b,
                                 func=AF.Sigmoid, bias=b2t[:, 0:1])
            ssum = small_pool.tile([1, P], FP32, tag="ssum", name="ssum")
            nc.vector.tensor_add(out=ssum, in0=sig[:, 0:P], in1=sig[:, P:2 * P])
            for b in range(2, B):
                nc.vector.tensor_add(out=ssum, in0=ssum,
                                     in1=sig[:, b * P:(b + 1) * P])
            # mask: scores > thr <=> ssum > B*thr (exact: x0.125 is a
            # power-of-2 scale). conft2 = conf * (LAMBDA/B) was computed
            # off the critical path right after the const DMA.
            mask = small_pool.tile([1, P], FP32, tag="mask", name="mask")
            nc.vector.tensor_scalar(out=mask, in0=ssum,
                                    scalar1=float(B) * SIM_THRESHOLD,
                                    scalar2=None, op0=ALU.is_gt)
            sc2 = small_pool.tile([1, P], FP32, tag="sc2", name="sc2")
            nc.vector.tensor_mul(out=sc2, in0=ssum, in1=conft2)
            w_vec = small_pool.tile([1, P], FP32, tag="w_vec", name="w_vec")
            nc.vector.tensor_mul(out=w_vec, in0=sc2, in1=mask)
            # broadcast w to all 128 partitions via rank-1 matmul
            wbc = psB.tile([128, P], FP32, tag="t2", bufs=2, name="wbc")
            nc.tensor.matmul(wbc, lhsT=ones, rhs=w_vec, start=True, stop=True)
            wsb = small_pool.tile([128, P], FP32, tag="wsb", name="wsb")
            nc.scalar.activation(out=wsb, in_=wbc, func=AF.Copy)

            # ---- memory-bound phase: acc = sum_p w[p]*bias[p] on Vector,
            # column-split so the left tail overlaps the right stream.
            # Chain merges run on the (idle) Pool engine; the attn adds
            # happen inside the SWDGE accum-DMA, so DVE's instruction
            # stream is pure bias STTs and never falls further behind ----
            for h in range(2):
                csl = slice(h * HALF, (h + 1) * HALF)
                # 8 separate accs: sharing tags across halves creates a
                # false WAR that blocks the right chain-0 (Pool) behind
                # the whole left tail
                acc = [acc_pool.tile([128, HALF], FP32, tag=f"ac{h}_{c}",
                                     name=f"ac{h}_{c}") for c in range(4)]
                for p in range(P):
                    bt = bias_tiles[(h, p)]
                    ci = p // CHAIN
                    w_ap = wsb[:, p:p + 1]
                    if ci == 0:
                        # chain 0 runs on ACT (in-place scale, private SBUF
                        # ports) + Pool (accumulate): ~18% of the stream
                        # comes off the Vector engine, which otherwise
                        # starts ~36us behind the arrivals and never
                        # catches up. Same two-rounded per-element float
                        # sequence as the DVE STT path.
                        nc.scalar.activation(out=bt, in_=bt, func=AF.Copy,
                                             scale=w_ap)
                        if p == 0:
                            nc.gpsimd.tensor_copy(out=acc[0], in_=bt)
                        else:
                            nc.gpsimd.tensor_add(out=acc[0], in0=acc[0],
                                                 in1=bt)
                    elif p % CHAIN == 0:
                        nc.vector.tensor_scalar_mul(out=acc[ci], in0=bt,
                                                    scalar1=w_ap)
                    else:
                        nc.vector.scalar_tensor_tensor(out=acc[ci], in0=bt,
                                                       scalar=w_ap, in1=acc[ci],
                                                       op0=ALU.mult, op1=ALU.add)

                # ---- tail: the left stores go out on the SWDGE (gpsimd)
                # ring — a separate issue resource — so a store waiting on
                # a left add can never head-of-line block the right-half
                # load stream on the HWDGE ring ----
                if h == 1:
                    for b in range(B):
                        nc.gpsimd.dma_start(out=out_s[b][:, 0:HALF],
                                            in_=attns[(0, b)])
                # merge tree with the same operand pairs as the baseline
                # ((0+1),(2+3),(0+2)) so per-element numerics match, then
                # DVE adds into the attn tiles
                nc.vector.tensor_add(out=acc[0], in0=acc[0], in1=acc[1])
                nc.vector.tensor_add(out=acc[2], in0=acc[2], in1=acc[3])
                nc.vector.tensor_add(out=acc[0], in0=acc[0], in1=acc[2])
                for b in range(B):
                    at = attns[(h, b)]
                    nc.vector.tensor_add(out=at, in0=at, in1=acc[0])
                    if h == 1:
                        nc.scalar.dma_start(out=out_s[b][:, csl], in_=at)

    # TRN2 matmul supports only one embedded semaphore wait; split the
    # extras onto InstEventSemaphore instructions (same pass Bacc runs).
    bass_rust.generate_event_semaphores(nc)
    return nc


def _get_nc() -> bass.Bass:
    if "nc" not in _NC_CACHE:
        _NC_CACHE["nc"] = _build_nc()
    return _NC_CACHE["nc"]


def _prep_in_maps(attention_scores, context, triggers, biases, confidences,
                  W1, b1, W2, b2):
    f32 = np.float32
    W1 = np.asarray(W1, dtype=f32)
    # [r, ch*4096 + kt*512 + c'] = W1half[kt*128 + r, ch*512 + c']
    w1hi_h = np.ascontiguousarray(
        W1[C:].reshape(8, 128, 2, 512).transpose(1, 2, 0, 3).reshape(128, 8192))
    w1lo_h = np.ascontiguousarray(
        W1[:C].reshape(8, 128, 2, 512).transpose(1, 2, 0, 3).reshape(128, 8192))
    trigp_h = (np.asarray(triggers, dtype=f32).T.reshape(8, 128, P)
               .transpose(1, 0, 2).reshape(128, 8 * P))
    ctxp_h = (np.asarray(context, dtype=f32).T.reshape(8, 128, B)
              .transpose(1, 0, 2).reshape(128, 8 * B))
    w2r_h = np.asarray(W2, dtype=f32).reshape(8, 128).T
    constsA_h = np.ascontiguousarray(
        np.concatenate([trigp_h, ctxp_h, w2r_h], axis=1))
    constsB_h = np.ascontiguousarray(
        np.concatenate([np.asarray(b1, dtype=f32).reshape(1, C),
                        np.asarray(confidences, dtype=f32).reshape(1, P),
                        np.asarray(b2, dtype=f32).reshape(1, 1)], axis=1))
    attention_scores = np.asarray(attention_scores, dtype=f32)
    biases = np.asarray(biases, dtype=f32)
    in_maps = []
    for r in range(NCORES):
        rows = slice(r * ROWS, (r + 1) * ROWS)
        in_maps.append({
            "bias_s": np.ascontiguousarray(biases[:, rows, :]),
            "attn_s": np.ascontiguousarray(attention_scores[:, rows, :]),
            "w1hi": w1hi_h,
            "w1lo": w1lo_h,
            "constsA": constsA_h,
            "constsB": constsB_h,
        })
    return in_maps


def run(trace=False, **inputs):
    nc = _get_nc()
    in_maps = _prep_in_maps(**inputs)
    res = run_bass_kernel_spmd(nc, in_maps, core_ids=list(range(NCORES)),
                               trace=trace)
    out = np.concatenate([np.asarray(res.results[r]["out_s"])
                          for r in range(NCORES)], axis=1)
    return out.astype(np.float32), res


def kernel(**inputs) -> np.ndarray:
    out, _ = run(trace=False, **inputs)
    return out
